# revision 1
# baseline (speedup 1.0000x reference)
"""Trainium2 Bass kernel for ClassCentersEMA (vq_codebook).

Reference semantics (B=16384, D=1024, C=512):
    feats_n   = feats / max(||feats||_row, eps)
    counts    = targets.sum(0)                       # [C]
    class_sums= targets^T @ feats_n                  # [C, D]
    mask      = counts > 0
    means     = class_sums / max(counts, 1)          # rows 0 where !mask
    new       = !initialized & mask
    base      = where(new, means, centers)
    blended   = 0.9*base + 0.1*means
    upd       = where(mask, blended, centers)
    out       = where(mask, upd / max(||upd||, eps), upd)

Distribution: data-parallel over B across 8 cores. Each core computes
local [C] count and [C, D] class-sum partials; ReduceScatters hand core
r the 64 classes [64r, 64r+64); the epilogue runs on that shard and the
host concatenates the 8 [64, 1024] outputs.

Matmul precision: split-precision bf16. The normalized features are
decomposed as feats_n ~= hi + lo with hi = bf16(feats_n) and
lo = bf16(feats_n - hi); class_sums = T^T@hi + T^T@lo accumulated in
the same fp32 PSUM group. targets are exactly representable in bf16,
so every product is exact and the only error is the ~2^-17 relative
decomposition error — better than fp32/fp32r matmul paths and ~2x
faster than fp32r on the PE. counts accumulate on the PE too
(ones^T @ targets per k-tile into a ninth PSUM region).

Schedule: two D-half passes. Pass a streams feats/targets from HBM
(normalizing + splitting rows on the fly; hi/lo/targets tiles stay
resident in SBUF) and computes class_sums[:, 0:512]; its ReduceScatter
and the tiny counts ReduceScatter overlap pass b, which computes
class_sums[:, 512:1024] from the resident tiles. Only the second
(1 MiB) ReduceScatter plus a short epilogue sit on the critical tail.

The epilogue is algebraically folded to
    upd = a_c * centers + b_c * class_sums
with per-class scalars  (a, b):
    !mask:            (1.0, 0)
    mask & inited:    (0.9, 0.1/counts)
    mask & !inited:   (0.0, 1.0/counts)
followed by a masked renormalize. All count-derived scalars are
computed as soon as the counts ReduceScatter lands, off the tail.
"""

import numpy as np

import concourse.bass as bass
import concourse.mybir as mybir
import concourse.tile as tile
from concourse import bacc
from concourse.bass_utils import run_bass_kernel_spmd

F32 = mybir.dt.float32
BF16 = mybir.dt.bfloat16
AF = mybir.ActivationFunctionType
ALU = mybir.AluOpType

NCORES = 8
B, D, C = 16384, 1024, 512
BL = B // NCORES          # 2048 rows per core
KT = BL // 128            # 16 k-tiles of 128
CL = C // NCORES          # 64 classes per core after ReduceScatter
MOM = 0.9
EPS = 1e-12


def build_nc(niters=1):
    """niters>1 unrolls the whole kernel body N times in one NEFF —
    used only for timing (slope over N isolates device exec time)."""
    nc = bacc.Bacc("TRN2", target_bir_lowering=False, debug=False,
                   num_devices=NCORES)

    feats = nc.dram_tensor("feats", [BL, D], F32, kind="ExternalInput")
    targets = nc.dram_tensor("targets", [BL, C], F32, kind="ExternalInput")
    centers = nc.dram_tensor("centers", [CL, D], F32, kind="ExternalInput")
    inited = nc.dram_tensor("inited", [CL, 1], F32, kind="ExternalInput")
    out = nc.dram_tensor("out", [CL, D], F32, kind="ExternalOutput")

    rg = [list(range(NCORES))]

    with tile.TileContext(nc) as tc:
        with (
            tc.tile_pool(name="dram", bufs=1, space="DRAM") as dram,
            tc.tile_pool(name="ftp", bufs=4) as ftp,
            tc.tile_pool(name="res", bufs=1) as res,
            tc.tile_pool(name="sq", bufs=2) as sqp,
            tc.tile_pool(name="small", bufs=6) as small,
            tc.tile_pool(name="single", bufs=1) as single,
            tc.tile_pool(name="cs", bufs=4) as csp,
            tc.tile_pool(name="psum", bufs=8, space="PSUM") as ppool,
            tc.tile_pool(name="epi", bufs=1) as epi,
        ):
            io = dict(feats=feats, targets=targets, centers=centers,
                      inited=inited, out=out)
            bounces = dict(
                rs_in_a=dram.tile([NCORES, CL + 1, 512], F32,
                                  name="rs_in_a"),
                rs_out_a=dram.tile([CL + 1, 512], F32, name="rs_out_a"),
                rs_in_b=dram.tile([C, 512], F32, name="rs_in_b"),
                rs_out_b=dram.tile([CL, 512], F32, name="rs_out_b"),
            )

            ones = single.tile([128, 1], BF16, name="ones")
            nc.vector.memset(ones[:], 1.0)
            eps2 = single.tile([128, 1], F32, name="eps2")
            nc.vector.memset(eps2[:], EPS * EPS)
            zrow = single.tile([1, 512 - CL], F32, name="zrow")
            nc.vector.memset(zrow[:], 0.0)
            consts = dict(ones=ones, eps2=eps2, zrow=zrow)

            pools = dict(ftp=ftp, res=res, sqp=sqp, small=small, csp=csp,
                         ppool=ppool, epi=epi)
            for _ in range(niters):
                _emit_iteration(nc, tc, io, bounces, consts, pools, rg)

    nc.compile()
    return nc


def _emit_iteration(nc, tc, io, bb, consts, pools, rg):
    feats, targets = io["feats"], io["targets"]
    centers, inited, out = io["centers"], io["inited"], io["out"]
    ones = consts["ones"]
    eps2 = consts["eps2"]
    zrow = consts["zrow"]
    ftp, res, sqp = pools["ftp"], pools["res"], pools["sqp"]
    small, csp, ppool, epi = (pools["small"], pools["csp"], pools["ppool"],
                              pools["epi"])

    # epilogue inputs that depend on nothing — issue DMAs up front
    ctr = epi.tile([CL, D], F32, tag="ctr")
    nc.sync.dma_start(ctr[:], centers[:])
    ini = epi.tile([CL, 1], F32, tag="ini")
    nc.sync.dma_start(ini[:], inited[:])

    # PSUM: pass a banks + the counts accumulator (5 of 8 slots); the
    # 4 pass-b banks are allocated at pass-b start and briefly wait for
    # pass-a drains
    ps_a = [ppool.tile([128, 512], F32, tag="acc", name=f"ps_a{c}")
            for c in range(4)]
    cps = ppool.tile([1, C], F32, tag="acc", name="cps")

    # DMA 2 k-tiles per transfer (~1 MiB): partition p, slot a holds
    # DRAM row (2q+a)*128 + p
    feats_t = feats.rearrange("(a p) d -> p a d", p=128)
    targets_t = targets.rearrange("(a p) c -> p a c", p=128)

    # ---- pass a: stream + normalize/split (resident) + matmul D[0:512] ----
    his, los, tgs = [], [], []
    for q in range(KT // 2):
        ft2 = ftp.tile([128, 2, D], F32, tag="ft")
        nc.sync.dma_start(ft2[:], feats_t[:, 2 * q:2 * q + 2, :])
        # targets are exactly 0/1 -> the f32->bf16 cast DMA (SWDGE) is exact
        tg2 = res.tile([128, 2, C], BF16, tag="tg", bufs=KT // 2,
                       name=f"tg{q}")
        nc.gpsimd.dma_start(tg2[:], targets_t[:, 2 * q:2 * q + 2, :])
        tgs.append(tg2)
        # row norms for the pair, batched: one Square+accum per sub-tile,
        # then a single sqrt and a single reciprocal on [128, 2] —
        # halves the small-op dispatch/drain overhead on ACT and DVE.
        # sqrt(ssq + eps^2) == max(sqrt(ssq), eps) for any non-degenerate
        # row, and exactly eps for a zero row
        ssq2 = small.tile([128, 2], F32, tag="ssq")
        for a in range(2):
            sq = sqp.tile([128, D], F32, tag="sq")
            nc.scalar.activation(sq[:], ft2[:, a, :], AF.Square,
                                 accum_out=ssq2[:, a:a + 1])
        nrm2 = small.tile([128, 2], F32, tag="nrm")
        nc.scalar.activation(nrm2[:], ssq2[:], AF.Sqrt, bias=eps2[:])
        rcp2 = small.tile([128, 2], F32, tag="rcp")
        nc.vector.reciprocal(rcp2[:], nrm2[:])
        for a in range(2):
            k = 2 * q + a
            ft = ft2[:, a, :]
            tg = tg2[:, a, :]
            rcp = rcp2[:, a:a + 1]
            # split-precision normalized feats, resident for pass b:
            # hi = bf16(ft*rcp);  lo = bf16(ft*rcp - hi)
            # hi alternates DVE/ACT by parity to balance the engines
            hi = res.tile([128, D], BF16, tag="hi", bufs=KT, name=f"hi{k}")
            if k % 2 == 0:
                nc.vector.tensor_scalar_mul(hi[:], ft, rcp[:])
            else:
                nc.scalar.activation(hi[:], ft, AF.Copy, scale=rcp[:])
            lo = res.tile([128, D], BF16, tag="lo", bufs=KT, name=f"lo{k}")
            nc.vector.scalar_tensor_tensor(lo[:], ft, rcp[:], hi[:],
                                           op0=ALU.mult, op1=ALU.subtract)
            his.append(hi)
            los.append(lo)

            # counts partial on the PE: cps += ones^T @ targets_k
            nc.tensor.matmul(cps[:], ones[:], tg,
                             start=(k == 0), stop=(k == KT - 1))
            # class_sums partial D[0:512]
            for c in range(4):
                lhs = tg[:, c * 128:(c + 1) * 128]
                nc.tensor.matmul(ps_a[c][:], lhs, hi[:, 0:512],
                                 start=(k == 0), stop=False)
                nc.tensor.matmul(ps_a[c][:], lhs, lo[:, 0:512],
                                 start=False, stop=(k == KT - 1))

    # drain pass-a PSUM into the slabbed bounce: slab r carries the 64
    # class rows for rank r plus a counts row, so one ReduceScatter
    # moves both (one less collective floor on the queue)
    cnt_sb = small.tile([1, C], F32, tag="cnt_sb")
    nc.vector.tensor_copy(cnt_sb[:], cps[:])
    for r in range(NCORES):
        nc.sync.dma_start(bb["rs_in_a"][r, CL:CL + 1, 0:CL],
                          cnt_sb[0:1, r * CL:(r + 1) * CL])
        nc.sync.dma_start(bb["rs_in_a"][r, CL:CL + 1, CL:512], zrow[:])
    for c in range(4):
        cs_sb = csp.tile([128, 512], F32, tag="cs_sb")
        if c % 2 == 0:
            nc.vector.tensor_copy(cs_sb[:], ps_a[c][:])
        else:
            nc.scalar.copy(cs_sb[:], ps_a[c][:])
        nc.sync.dma_start(bb["rs_in_a"][2 * c, 0:CL, :], cs_sb[0:CL, :])
        nc.sync.dma_start(bb["rs_in_a"][2 * c + 1, 0:CL, :],
                          cs_sb[CL:128, :])
    nc.gpsimd.collective_compute(
        "ReduceScatter", ALU.add, replica_groups=rg,
        ins=[bb["rs_in_a"][:].opt()], outs=[bb["rs_out_a"][:].opt()])

    # ---- pass b: matmul D[512:1024] from resident tiles ----
    # c-major so bank c drains (and its bounce DMA issues) while banks
    # c+1.. are still accumulating — only the last drain sits on the tail
    ps_b = [ppool.tile([128, 512], F32, tag="acc", name=f"ps_b{c}")
            for c in range(4)]
    for c in range(4):
        for k in range(KT):
            tg = tgs[k // 2][:, k % 2, :]
            lhs = tg[:, c * 128:(c + 1) * 128]
            nc.tensor.matmul(ps_b[c][:], lhs, his[k][:, 512:1024],
                             start=(k == 0), stop=False)
            nc.tensor.matmul(ps_b[c][:], lhs, los[k][:, 512:1024],
                             start=False, stop=(k == KT - 1))
        cs_sb = csp.tile([128, 512], F32, tag="cs_sb")
        if c % 2 == 0:
            nc.vector.tensor_copy(cs_sb[:], ps_b[c][:])
        else:
            nc.scalar.copy(cs_sb[:], ps_b[c][:])
        nc.sync.dma_start(bb["rs_in_b"][c * 128:(c + 1) * 128, :], cs_sb[:])
    nc.gpsimd.collective_compute(
        "ReduceScatter", ALU.add, replica_groups=rg,
        ins=[bb["rs_in_b"][:].opt()], outs=[bb["rs_out_b"][:].opt()])

    # ---- epilogue on this core's CL classes ----
    # count-derived per-class scalars: ready as soon as the tiny RS lands
    cnt = epi.tile([CL, 1], F32, tag="cnt")
    nc.sync.dma_start(
        cnt[:],
        bb["rs_out_a"][CL:CL + 1, 0:CL].rearrange("a c -> (a c)").unsqueeze(1))
    mask = epi.tile([CL, 1], F32, tag="mask")
    nc.vector.tensor_scalar_min(mask[:], cnt[:], 1.0)
    omask = epi.tile([CL, 1], F32, tag="omask")
    nc.vector.tensor_scalar(omask[:], mask[:], -1.0, 1.0,
                            op0=ALU.mult, op1=ALU.add)
    inv = epi.tile([CL, 1], F32, tag="inv")
    nc.vector.tensor_scalar_max(inv[:], cnt[:], 1.0)
    nc.vector.reciprocal(inv[:], inv[:])
    # new01 = (1 - inited) * mask
    new01 = epi.tile([CL, 1], F32, tag="new01")
    nc.vector.tensor_scalar(new01[:], ini[:], -1.0, 1.0,
                            op0=ALU.mult, op1=ALU.add)
    nc.vector.tensor_mul(new01[:], new01[:], mask[:])
    # b = mask * (0.1 + 0.9*new01); a = 1 - b; bp = b / max(cnt,1)
    bco = epi.tile([CL, 1], F32, tag="bco")
    nc.vector.tensor_scalar(bco[:], new01[:], MOM, 1.0 - MOM,
                            op0=ALU.mult, op1=ALU.add)
    nc.vector.tensor_mul(bco[:], bco[:], mask[:])
    aco = epi.tile([CL, 1], F32, tag="aco")
    nc.vector.tensor_scalar(aco[:], bco[:], -1.0, 1.0,
                            op0=ALU.mult, op1=ALU.add)
    nc.vector.tensor_mul(bco[:], bco[:], inv[:])
    # base = a*centers, ready before the big RS lands
    upd = epi.tile([CL, D], F32, tag="upd")
    nc.vector.tensor_scalar_mul(upd[:], ctr[:], aco[:])

    cs = epi.tile([CL, D], F32, tag="cs")
    nc.sync.dma_start(cs[:, 0:512], bb["rs_out_a"][0:CL, :])
    nc.sync.dma_start(cs[:, 512:1024], bb["rs_out_b"][:])

    # upd += bp*class_sums  (half a: off the tail; half b: the tail)
    nc.vector.scalar_tensor_tensor(upd[:, 0:512], cs[:, 0:512], bco[:],
                                   upd[:, 0:512], op0=ALU.mult, op1=ALU.add)
    nc.vector.scalar_tensor_tensor(upd[:, 512:1024], cs[:, 512:1024], bco[:],
                                   upd[:, 512:1024],
                                   op0=ALU.mult, op1=ALU.add)

    # masked renormalize; ssq of half a computes off the tail
    usq = epi.tile([CL, D], F32, tag="usq")
    ussq_a = epi.tile([CL, 1], F32, tag="ussq_a")
    nc.scalar.activation(usq[:, 0:512], upd[:, 0:512], AF.Square,
                         accum_out=ussq_a[:])
    ussq_b = epi.tile([CL, 1], F32, tag="ussq_b")
    nc.scalar.activation(usq[:, 512:1024], upd[:, 512:1024], AF.Square,
                         accum_out=ussq_b[:])
    ussq = epi.tile([CL, 1], F32, tag="ussq")
    nc.vector.tensor_add(ussq[:], ussq_a[:], ussq_b[:])
    unrm = epi.tile([CL, 1], F32, tag="unrm")
    nc.scalar.activation(unrm[:], ussq[:], AF.Sqrt, bias=eps2[0:CL, :])
    urcp = epi.tile([CL, 1], F32, tag="urcp")
    nc.vector.reciprocal(urcp[:], unrm[:])
    # rfin = mask*urcp + (1-mask), one fused tensor_scalar
    nc.vector.tensor_scalar(urcp[:], mask[:], urcp[:], omask[:],
                            op0=ALU.mult, op1=ALU.add)

    ov = epi.tile([CL, D], F32, tag="ov")
    nc.vector.tensor_scalar_mul(ov[:], upd[:], urcp[:])
    nc.sync.dma_start(out[:], ov[:])


_NC_CACHE = None


def _get_nc():
    global _NC_CACHE
    if _NC_CACHE is None:
        _NC_CACHE = build_nc()
    return _NC_CACHE


def run_spmd(feats, targets, centers, initialized, **kw):
    feats = np.ascontiguousarray(np.asarray(feats, dtype=np.float32))
    targets = np.ascontiguousarray(np.asarray(targets, dtype=np.float32))
    centers = np.ascontiguousarray(np.asarray(centers, dtype=np.float32))
    init_f = np.asarray(initialized).astype(np.float32).reshape(C, 1)
    assert feats.shape == (B, D) and targets.shape == (B, C)
    assert centers.shape == (C, D)

    nc = _get_nc()
    in_maps = []
    for r in range(NCORES):
        in_maps.append({
            "feats": feats[r * BL:(r + 1) * BL],
            "targets": targets[r * BL:(r + 1) * BL],
            "centers": np.ascontiguousarray(centers[r * CL:(r + 1) * CL]),
            "inited": np.ascontiguousarray(init_f[r * CL:(r + 1) * CL]),
        })
    res = run_bass_kernel_spmd(nc, in_maps, core_ids=list(range(NCORES)), **kw)
    out = np.concatenate([res.results[r]["out"] for r in range(NCORES)], axis=0)
    return out.astype(np.float32), res


def kernel(feats, targets, centers, initialized):
    out, _ = run_spmd(feats, targets, centers, initialized)
    return out



# revision 3
# speedup vs baseline: 1.7223x; 1.7223x over previous
"""Trainium2 Bass kernel for ClassCentersEMA (vq_codebook) — v2.

Reference semantics (B=16384, D=1024, C=512):
    feats_n   = feats / max(||feats||_row, eps)
    counts    = targets.sum(0)                       # [C]
    class_sums= targets^T @ feats_n                  # [C, D]
    mask      = counts > 0
    means     = class_sums / max(counts, 1)          # rows 0 where !mask
    new       = !initialized & mask
    base      = where(new, means, centers)
    blended   = 0.9*base + 0.1*means
    upd       = where(mask, blended, centers)
    out       = where(mask, upd / max(||upd||, eps), upd)

Distribution: data-parallel over B across 8 cores; one ReduceScatter
hands core r its 64 classes; epilogue on the shard; host concatenates.

vs the original two-ReduceScatter split-precision design (131.8us ->
56.6us measured):
  - bf16 single-precision matmul (rel err ~2e-3, gate is 2e-2)
    instead of split-precision hi/lo: halves PE work.
  - The 1/||f|| row scale is folded into TARGETS (512 cols) instead of
    feats (1024 cols); feats only needs a data-independent bf16 cast,
    so the last-tile dependency chain is shorter and DVE does less.
  - Single fused pass over all of D (8 PSUM banks).
  - counts = ones-rhs matmuls (free size 1) into a PSUM bank recycled
    after the stream: ~64 cycles of PE instead of 16 free-512 matmuls.
  - The 8-way cross-core reduction uses an f32 AllToAll of the
    [8, 65, 1024] partials slab (64 class rows + 1 counts row per
    rank) plus a local DVE/Pool pairwise tree-sum, NOT a
    ReduceScatter: the RS ALU ring measures ~47us on this HW while
    the point-to-point scatter + ~4us of local adds is far cheaper.
    (A bf16-payload variant measured WORSE end-to-end — the 2-byte
    dtype appears to fall off the fast collective path.)
"""

import numpy as np

import concourse.bass as bass
import concourse.mybir as mybir
import concourse.tile as tile
from concourse import bacc
from concourse.bass_utils import run_bass_kernel_spmd

F32 = mybir.dt.float32
BF16 = mybir.dt.bfloat16
AF = mybir.ActivationFunctionType
ALU = mybir.AluOpType

NCORES = 8
B, D, C = 16384, 1024, 512
BL = B // NCORES          # 2048 rows per core
KT = BL // 128            # 16 k-tiles of 128
CL = C // NCORES          # 64 classes per core after ReduceScatter
MOM = 0.9
EPS = 1e-12


def build_nc(niters=1, use_coll=True):
    """niters>1 unrolls the whole kernel body N times in one NEFF —
    used only for timing (slope over N isolates device exec time).
    use_coll=False replaces the ReduceScatter with a local DMA copy of
    the rank-0 slab (wrong output; isolates collective cost)."""
    nc = bacc.Bacc("TRN2", target_bir_lowering=False, debug=False,
                   num_devices=NCORES)

    feats = nc.dram_tensor("feats", [BL, D], F32, kind="ExternalInput")
    targets = nc.dram_tensor("targets", [BL, C], F32, kind="ExternalInput")
    centers = nc.dram_tensor("centers", [CL, D], F32, kind="ExternalInput")
    inited = nc.dram_tensor("inited", [CL, 1], F32, kind="ExternalInput")
    out = nc.dram_tensor("out", [CL, D], F32, kind="ExternalOutput")

    rg = [list(range(NCORES))]

    with tile.TileContext(nc) as tc:
        with (
            tc.tile_pool(name="dram", bufs=1, space="DRAM") as dram,
            tc.tile_pool(name="ftp", bufs=3) as ftp,
            tc.tile_pool(name="fbp", bufs=3) as fbp,
            tc.tile_pool(name="tgp", bufs=1) as tgp,
            tc.tile_pool(name="tsp", bufs=3) as tsp,
            tc.tile_pool(name="sq", bufs=2) as sqp,
            tc.tile_pool(name="small", bufs=6) as small,
            tc.tile_pool(name="single", bufs=1) as single,
            tc.tile_pool(name="cs", bufs=4) as csp,
            tc.tile_pool(name="psum", bufs=8, space="PSUM") as ppool,
            tc.tile_pool(name="epi", bufs=1) as epi,
        ):
            io = dict(feats=feats, targets=targets, centers=centers,
                      inited=inited, out=out)
            # AllToAll instead of ReduceScatter: pure point-to-point
            # scatter (no ALU ring); the 8-way reduction happens locally
            # on DVE/Pool after the scatter lands.
            bounces = dict(
                rs_in=dram.tile([NCORES, CL + 1, D], F32, name="rs_in"),
                rs_out=dram.tile([NCORES, CL + 1, D], F32, name="rs_out"),
            )

            ones = single.tile([128, 1], BF16, name="ones")
            nc.vector.memset(ones[:], 1.0)
            eps2 = single.tile([128, 1], F32, name="eps2")
            nc.vector.memset(eps2[:], EPS * EPS)
            consts = dict(ones=ones, eps2=eps2)

            pools = dict(ftp=ftp, fbp=fbp, tgp=tgp, tsp=tsp, sqp=sqp,
                         small=small, csp=csp, ppool=ppool, epi=epi)
            for _ in range(niters):
                _emit_iteration(nc, tc, io, bounces, consts, pools, rg,
                                use_coll)

    nc.compile()
    return nc


def _emit_iteration(nc, tc, io, bb, consts, pools, rg, use_coll):
    feats, targets = io["feats"], io["targets"]
    centers, inited, out = io["centers"], io["inited"], io["out"]
    ones, eps2 = consts["ones"], consts["eps2"]
    ftp, fbp, tgp, tsp = pools["ftp"], pools["fbp"], pools["tgp"], pools["tsp"]
    sqp, small, csp = pools["sqp"], pools["small"], pools["csp"]
    ppool, epi = pools["ppool"], pools["epi"]

    # epilogue inputs that depend on nothing — issue DMAs up front
    ctr = epi.tile([CL, D], F32, tag="ctr")
    nc.scalar.dma_start(ctr[:], centers[:])
    ini = epi.tile([CL, 1], F32, tag="ini")
    nc.scalar.dma_start(ini[:], inited[:])

    # 8 PSUM banks: (c-block, D-half) accumulated over all 16 k-tiles
    ps = [ppool.tile([128, 512], F32, tag="acc", name=f"ps{i}")
          for i in range(8)]

    # DMA 2 k-tiles per transfer (~1 MiB): partition p, slot a holds
    # DRAM row (2q+a)*128 + p
    feats_t = feats.rearrange("(a p) d -> p a d", p=128)
    targets_t = targets.rearrange("(a p) c -> p a c", p=128)

    # ---- fused stream pass: normalize-scale targets + matmul all of D ----
    tgs_raw = []
    for q in range(KT // 2):
        ft2 = ftp.tile([128, 2, D], F32, tag="ft")
        nc.sync.dma_start(ft2[:], feats_t[:, 2 * q:2 * q + 2, :])
        # targets are exactly 0/1 -> the f32->bf16 cast DMA (SWDGE) is exact
        tg2 = tgp.tile([128, 2, C], BF16, tag="tg", bufs=KT // 2,
                       name=f"tg{q}")
        nc.gpsimd.dma_start(tg2[:], targets_t[:, 2 * q:2 * q + 2, :])
        tgs_raw.append(tg2)
        for a in range(2):
            k = 2 * q + a
            ft = ft2[:, a, :]
            # row norms: Square+accum then rsqrt(ssq + eps^2)
            # (== 1/max(||f||, eps) for any non-degenerate row; a zero
            # row gives 1/eps which multiplies a zero targets row -> 0)
            sq = sqp.tile([128, D], F32, tag="sq")
            ssq = small.tile([128, 1], F32, tag="ssq")
            nc.scalar.activation(sq[:], ft, AF.Square, accum_out=ssq[:])
            nrm = small.tile([128, 1], F32, tag="nrm")
            nc.scalar.activation(nrm[:], ssq[:], AF.Sqrt, bias=eps2[:])
            rcp = small.tile([128, 1], F32, tag="rcp")
            nc.vector.reciprocal(rcp[:], nrm[:])
            # feats bf16 cast (no dependency on the norm)
            fb = fbp.tile([128, D], BF16, tag="fb")
            nc.vector.tensor_copy(fb[:], ft)
            # row-scaled targets: tgs = bf16(tg / ||f||)
            tgs = tsp.tile([128, C], BF16, tag="tgs")
            nc.vector.tensor_scalar_mul(tgs[:], tg2[:, a, :], rcp[:])
            for c in range(4):
                lhs = tgs[:, c * 128:(c + 1) * 128]
                nc.tensor.matmul(ps[2 * c][:], lhs, fb[:, 0:512],
                                 start=(k == 0), stop=(k == KT - 1))
                nc.tensor.matmul(ps[2 * c + 1][:], lhs, fb[:, 512:1024],
                                 start=(k == 0), stop=(k == KT - 1))

    # ---- drain PSUM into the slab; counts into the first freed bank ----
    # slab rank r: rows 0..63 = class_sums for classes [64r, 64r+64),
    # row 64 cols 0:64 = counts (cols 64: garbage, reduced but unread)
    for i in range(8):
        c, h = i // 2, i % 2
        cs_sb = csp.tile([128, 512], F32, tag="cs_sb")
        if i % 2 == 0:
            nc.vector.tensor_copy(cs_sb[:], ps[i][:])
        else:
            nc.scalar.copy(cs_sb[:], ps[i][:])
        eng = nc.sync if i % 2 == 0 else nc.scalar
        eng.dma_start(bb["rs_in"][2 * c, 0:CL, 512 * h:512 * h + 512],
                      cs_sb[0:CL, :])
        eng.dma_start(bb["rs_in"][2 * c + 1, 0:CL, 512 * h:512 * h + 512],
                      cs_sb[CL:128, :])
        if i == 0:
            # counts: free-size-1 matmuls into the recycled bank
            cnt_ps = ppool.tile([128, 4], F32, tag="acc", name="cnt_ps")
            for k in range(KT):
                tg = tgs_raw[k // 2][:, k % 2, :]
                for c2 in range(4):
                    nc.tensor.matmul(
                        cnt_ps[:, c2:c2 + 1],
                        tg[:, c2 * 128:(c2 + 1) * 128], ones[:],
                        start=(k == 0), stop=(k == KT - 1))
    cnt_sb = small.tile([128, 4], F32, tag="cnt_sb")
    nc.vector.tensor_copy(cnt_sb[:], cnt_ps[:])
    for r in range(NCORES):
        nc.sync.dma_start(
            bb["rs_in"][r, CL:CL + 1, 0:CL]
            .rearrange("a c -> (a c)").unsqueeze(1),
            cnt_sb[(r % 2) * CL:(r % 2) * CL + CL, r // 2:r // 2 + 1])

    if use_coll:
        nc.gpsimd.collective_compute(
            "AllToAll", ALU.bypass, replica_groups=rg,
            ins=[bb["rs_in"][:].opt()], outs=[bb["rs_out"][:].opt()])
    else:
        # timing-only variant: local copy keeps the data dependency but
        # skips the collective (output is wrong)
        nc.sync.dma_start(bb["rs_out"][:], bb["rs_in"][:])

    # ---- local 8-way reduction of the scattered slabs ----
    # slabs stream in on both HWDGE queues; pairwise tree on DVE + Pool
    slabs = []
    for s in range(NCORES):
        sl = epi.tile([CL + 1, D], F32, tag=f"slab{s}", name=f"slab{s}")
        eng = nc.sync if s % 2 == 0 else nc.scalar
        eng.dma_start(sl[:], bb["rs_out"][s])
        slabs.append(sl)
    red = []
    for i in range(4):
        dst = slabs[2 * i]
        eng = nc.vector if i % 2 == 0 else nc.gpsimd
        eng.tensor_add(dst[:], dst[:], slabs[2 * i + 1][:])
        red.append(dst)
    nc.vector.tensor_add(red[0][:], red[0][:], red[1][:])
    nc.gpsimd.tensor_add(red[2][:], red[2][:], red[3][:])
    tot = red[0]
    nc.vector.tensor_add(tot[:], tot[:], red[2][:])
    cs = tot[0:CL, :]

    # counts: row 64 cols 0:64 of the summed slab -> [64,1] column,
    # bounced through DRAM (DRAM-side APs may rearrange freely)
    cnt_row = bb["rs_in"][0, CL:CL + 1, 0:CL]
    nc.sync.dma_start(cnt_row, tot[CL:CL + 1, 0:CL])
    cnt = epi.tile([CL, 1], F32, tag="cnt")
    nc.sync.dma_start(cnt[:],
                      cnt_row.rearrange("a c -> (a c)").unsqueeze(1))

    mask = epi.tile([CL, 1], F32, tag="mask")
    nc.vector.tensor_scalar_min(mask[:], cnt[:], 1.0)
    omask = epi.tile([CL, 1], F32, tag="omask")
    nc.vector.tensor_scalar(omask[:], mask[:], -1.0, 1.0,
                            op0=ALU.mult, op1=ALU.add)
    inv = epi.tile([CL, 1], F32, tag="inv")
    nc.vector.tensor_scalar_max(inv[:], cnt[:], 1.0)
    nc.vector.reciprocal(inv[:], inv[:])
    # new01 = (1 - inited) * mask
    new01 = epi.tile([CL, 1], F32, tag="new01")
    nc.vector.tensor_scalar(new01[:], ini[:], -1.0, 1.0,
                            op0=ALU.mult, op1=ALU.add)
    nc.vector.tensor_mul(new01[:], new01[:], mask[:])
    # b = mask * (0.1 + 0.9*new01); a = 1 - b; bp = b / max(cnt,1)
    bco = epi.tile([CL, 1], F32, tag="bco")
    nc.vector.tensor_scalar(bco[:], new01[:], MOM, 1.0 - MOM,
                            op0=ALU.mult, op1=ALU.add)
    nc.vector.tensor_mul(bco[:], bco[:], mask[:])
    aco = epi.tile([CL, 1], F32, tag="aco")
    nc.vector.tensor_scalar(aco[:], bco[:], -1.0, 1.0,
                            op0=ALU.mult, op1=ALU.add)
    nc.vector.tensor_mul(bco[:], bco[:], inv[:])

    # upd = a*centers + bp*class_sums, halves split DVE / Pool
    upd = epi.tile([CL, D], F32, tag="upd")
    nc.vector.tensor_scalar_mul(upd[:], ctr[:], aco[:])
    nc.vector.scalar_tensor_tensor(upd[:, 0:512], cs[:, 0:512], bco[:],
                                   upd[:, 0:512], op0=ALU.mult, op1=ALU.add)
    nc.vector.scalar_tensor_tensor(upd[:, 512:1024], cs[:, 512:1024], bco[:],
                                   upd[:, 512:1024],
                                   op0=ALU.mult, op1=ALU.add)

    # masked renormalize
    usq = epi.tile([CL, D], F32, tag="usq")
    ussq_a = epi.tile([CL, 1], F32, tag="ussq_a")
    nc.scalar.activation(usq[:, 0:512], upd[:, 0:512], AF.Square,
                         accum_out=ussq_a[:])
    ussq_b = epi.tile([CL, 1], F32, tag="ussq_b")
    nc.scalar.activation(usq[:, 512:1024], upd[:, 512:1024], AF.Square,
                         accum_out=ussq_b[:])
    ussq = epi.tile([CL, 1], F32, tag="ussq")
    nc.vector.tensor_add(ussq[:], ussq_a[:], ussq_b[:])
    unrm = epi.tile([CL, 1], F32, tag="unrm")
    nc.scalar.activation(unrm[:], ussq[:], AF.Sqrt, bias=eps2[0:CL, :])
    urcp = epi.tile([CL, 1], F32, tag="urcp")
    nc.vector.reciprocal(urcp[:], unrm[:])
    # rfin = mask*urcp + (1-mask), one fused tensor_scalar
    nc.vector.tensor_scalar(urcp[:], mask[:], urcp[:], omask[:],
                            op0=ALU.mult, op1=ALU.add)

    ov = epi.tile([CL, D], F32, tag="ov")
    nc.vector.tensor_scalar_mul(ov[:, 0:512], upd[:, 0:512], urcp[:])
    nc.scalar.activation(ov[:, 512:1024], upd[:, 512:1024], AF.Copy,
                         scale=urcp[:])
    nc.sync.dma_start(out[:], ov[:])


_NC_CACHE = None


def _get_nc():
    global _NC_CACHE
    if _NC_CACHE is None:
        _NC_CACHE = build_nc()
    return _NC_CACHE


def run_spmd(feats, targets, centers, initialized, **kw):
    feats = np.ascontiguousarray(np.asarray(feats, dtype=np.float32))
    targets = np.ascontiguousarray(np.asarray(targets, dtype=np.float32))
    centers = np.ascontiguousarray(np.asarray(centers, dtype=np.float32))
    init_f = np.asarray(initialized).astype(np.float32).reshape(C, 1)
    assert feats.shape == (B, D) and targets.shape == (B, C)
    assert centers.shape == (C, D)

    nc = _get_nc()
    in_maps = []
    for r in range(NCORES):
        in_maps.append({
            "feats": feats[r * BL:(r + 1) * BL],
            "targets": targets[r * BL:(r + 1) * BL],
            "centers": np.ascontiguousarray(centers[r * CL:(r + 1) * CL]),
            "inited": np.ascontiguousarray(init_f[r * CL:(r + 1) * CL]),
        })
    res = run_bass_kernel_spmd(nc, in_maps, core_ids=list(range(NCORES)), **kw)
    out = np.concatenate([res.results[r]["out"] for r in range(NCORES)], axis=0)
    return out.astype(np.float32), res


def kernel(feats, targets, centers, initialized):
    out, _ = run_spmd(feats, targets, centers, initialized)
    return out
